# Initial kernel scaffold
#
"""Trainium2 Bass kernel for nn_AggregateGCN (3-layer GCN, batched graph,
agent-node readout).

Math (reference): deg-normalized GraphConv x2 on top of a linear+relu input
projection, then a final projection of the 64 agent rows (nodes 0, N, 2N, ...).
Only the 64 agent rows of the last conv are read, so the exact computation
is the backward dependency cone:
  layer2 needs edges into the 64 agents (~2k edges -> ~2k distinct sources S1)
  layer1 needs edges into S1 (~64k edges), with per-edge h0 = relu(x@w_lin+b)
Degrees (in/out over ALL 4M edges) feed the symmetric normalization; the
host extracts integer degree counts + edge buckets (index-only preprocessing),
all float math runs on device.

Sharding: core c owns agents 8c..8c+7 and its full (replicated) cone ->
zero cross-device traffic; host concatenates the per-core [8, 64] outputs.

On device per core:
  - per-edge h0 rows via matmul chunks (lhsT = host-transposed x_e^T slice)
  - SpMM via selection-matrix matmuls accumulated in PSUM: S[e,d]=(dst_e==d)
  - degree norms folded in as per-partition ACT scales; biases as K=1
    rank-1 matmuls initializing the PSUM accumulation
"""
import sys

sys.path.insert(0, "/opt/trn_rl_repo")

import numpy as np
import concourse.bass as bass
import concourse.mybir as mybir
import concourse.tile as tile
from concourse.masks import make_identity

F32 = mybir.dt.float32
F32R = mybir.dt.float32r
AF = mybir.ActivationFunctionType
ALU = mybir.AluOpType

# problem constants (fixed by the spec)
B = 64          # graphs
NPG = 2048      # nodes per graph
TOTAL = B * NPG
IN_DIM = 128
HID = 256
EMB = 64
NCORES = 8
AGENTS_PER_CORE = B // NCORES      # 8
M1 = 384                           # padded S1 slots per core (3 halves of 128)
NHALF = M1 // 128                  # 3
P = 128

USE_F32R = True  # fast fp32 matmul mode; flip to False if numerics degrade


def _mm(ap, use_f32r):
    return ap.bitcast(F32R) if use_f32r else ap


def build_program(nch_per_half: int, use_f32r: bool = USE_F32R) -> bass.Bass:
    """One SPMD program; per-core data differs via in_maps."""
    nchunk = NHALF * nch_per_half
    ne = nchunk * P

    nc = bass.Bass()
    # per-core inputs
    xeT = nc.declare_dram_parameter("xeT", [P, ne], F32, isOutput=False)
    odeg_e = nc.declare_dram_parameter("odeg_e", [P, nchunk], F32, isOutput=False)
    dstl_e = nc.declare_dram_parameter("dstl_e", [P, nchunk], F32, isOutput=False)
    indeg1 = nc.declare_dram_parameter("indeg1", [P, NHALF], F32, isOutput=False)
    outdeg1 = nc.declare_dram_parameter("outdeg1", [P, NHALF], F32, isOutput=False)
    indeg2 = nc.declare_dram_parameter("indeg2", [AGENTS_PER_CORE, 1], F32, isOutput=False)
    a2t = nc.declare_dram_parameter("a2t", [M1, AGENTS_PER_CORE], F32, isOutput=False)
    # replicated weights
    wlin = nc.declare_dram_parameter("wlin", [IN_DIM, HID], F32, isOutput=False)
    blin = nc.declare_dram_parameter("blin", [1, HID], F32, isOutput=False)
    wc0 = nc.declare_dram_parameter("wc0", [HID, HID], F32, isOutput=False)
    bc0 = nc.declare_dram_parameter("bc0", [1, HID], F32, isOutput=False)
    wc1 = nc.declare_dram_parameter("wc1", [HID, HID], F32, isOutput=False)
    bc1 = nc.declare_dram_parameter("bc1", [1, HID], F32, isOutput=False)
    wemb = nc.declare_dram_parameter("wemb", [HID, EMB], F32, isOutput=False)
    bemb = nc.declare_dram_parameter("bemb", [1, EMB], F32, isOutput=False)
    iota_in = nc.declare_dram_parameter("iota_in", [P, P], F32, isOutput=False)
    out = nc.declare_dram_parameter("out", [AGENTS_PER_CORE, EMB], F32, isOutput=True)

    with tile.TileContext(nc) as tc:
        with (
            tc.tile_pool(name="const", bufs=1) as cp,
            tc.tile_pool(name="hs0p", bufs=4) as hs0p,
            tc.tile_pool(name="selp", bufs=4) as selp,
            tc.tile_pool(name="stage", bufs=1) as stp,
            tc.tile_pool(name="h0ps", bufs=2, space="PSUM") as h0psp,
            tc.tile_pool(name="aggps", bufs=2, space="PSUM") as aggpsp,
            tc.tile_pool(name="trps", bufs=2, space="PSUM") as trpsp,
            tc.tile_pool(name="mlpps", bufs=1, space="PSUM") as mlppsp,
        ):
            # ---- constants / per-core metadata ----
            xeT_t = cp.tile([P, ne], F32)
            nc.sync.dma_start(out=xeT_t[:], in_=xeT[:])
            wlin_t = cp.tile([IN_DIM, HID], F32)
            nc.sync.dma_start(out=wlin_t[:], in_=wlin[:])
            wc0_t = cp.tile([HID // P, P, HID], F32)
            nc.sync.dma_start(out=wc0_t[:], in_=wc0.rearrange("(c p) n -> c p n", p=P))
            wc1_t = cp.tile([HID // P, P, HID], F32)
            nc.sync.dma_start(out=wc1_t[:], in_=wc1.rearrange("(c p) n -> c p n", p=P))
            wemb_t = cp.tile([HID // P, P, EMB], F32)
            nc.sync.dma_start(out=wemb_t[:], in_=wemb.rearrange("(c p) n -> c p n", p=P))
            blin_t = cp.tile([1, HID], F32)
            nc.sync.dma_start(out=blin_t[:], in_=blin[:])
            bc0_t = cp.tile([1, HID], F32)
            nc.sync.dma_start(out=bc0_t[:], in_=bc0[:])
            bc1_t = cp.tile([1, HID], F32)
            nc.sync.dma_start(out=bc1_t[:], in_=bc1[:])
            bemb_t = cp.tile([1, EMB], F32)
            nc.sync.dma_start(out=bemb_t[:], in_=bemb[:])
            iota_t = cp.tile([P, P], F32)
            nc.sync.dma_start(out=iota_t[:], in_=iota_in[:])
            dstl_t = cp.tile([P, nchunk], F32)
            nc.sync.dma_start(out=dstl_t[:], in_=dstl_e[:])
            a2t_t = cp.tile([NHALF, P, AGENTS_PER_CORE], F32)
            nc.sync.dma_start(out=a2t_t[:], in_=a2t.rearrange("(c p) n -> c p n", p=P))

            ident_t = cp.tile([P, P], F32)
            make_identity(nc, ident_t[:])
            ones_t = cp.tile([1, P], F32)
            nc.vector.memset(ones_t[:], 1.0)

            # ---- degree -> norm scales (rsqrt(max(d,1))) ----
            def rsqrt_of(dram_ap, shape):
                t = cp.tile(shape, F32)
                nc.sync.dma_start(out=t[:], in_=dram_ap)
                nc.vector.tensor_scalar(
                    out=t[:], in0=t[:], scalar1=1.0, scalar2=None, op0=ALU.max
                )
                nc.vector.reciprocal(out=t[:], in_=t[:])
                nc.scalar.activation(t[:], t[:], AF.Sqrt)
                return t

            se_t = rsqrt_of(odeg_e[:], [P, nchunk])          # per-edge out_norm
            in1_t = rsqrt_of(indeg1[:], [P, NHALF])          # per-S1-slot in_norm
            on1_t = rsqrt_of(outdeg1[:], [P, NHALF])         # per-S1-slot out_norm
            in2_t = rsqrt_of(indeg2[:], [AGENTS_PER_CORE, 1])  # per-agent in_norm

            # ---- stage A: per half, accumulate agg1 then h1 ----
            hs1_t = stp.tile([NHALF, P, HID], F32)   # node-major, lives to stage B
            for h in range(NHALF):
                agg_ps = aggpsp.tile([P, HID], F32, tag="agg")
                for j in range(nch_per_half):
                    c = h * nch_per_half + j
                    h0_ps = h0psp.tile([P, HID], F32, tag="h0")
                    nc.tensor.matmul(
                        out=h0_ps[:], lhsT=_mm(ones_t[:], use_f32r),
                        rhs=_mm(blin_t[:], use_f32r), start=True, stop=False,
                    )
                    nc.tensor.matmul(
                        out=h0_ps[:],
                        lhsT=_mm(xeT_t[:, c * P:(c + 1) * P], use_f32r),
                        rhs=_mm(wlin_t[:], use_f32r),
                        start=False, stop=True,
                    )
                    hs0_t = hs0p.tile([P, HID], F32, tag="hs0")
                    nc.scalar.activation(
                        hs0_t[:], h0_ps[:], AF.Relu, scale=se_t[:, c:c + 1]
                    )
                    s_t = selp.tile([P, P], F32, tag="sel")
                    nc.vector.tensor_tensor(
                        out=s_t[:],
                        in0=dstl_t[:, c:c + 1].to_broadcast([P, P]),
                        in1=iota_t[:],
                        op=ALU.is_equal,
                    )
                    nc.tensor.matmul(
                        out=agg_ps[:], lhsT=_mm(s_t[:], use_f32r),
                        rhs=_mm(hs0_t[:], use_f32r),
                        start=(j == 0), stop=(j == nch_per_half - 1),
                    )
                # da = in_norm * agg  (row scale), then transpose for next matmul
                da_t = hs0p.tile([P, HID], F32, tag="da")
                nc.scalar.activation(
                    da_t[:], agg_ps[:], AF.Copy, scale=in1_t[:, h:h + 1]
                )
                h1_ps = trpsp.tile([P, HID], F32, tag="h1")
                nc.tensor.matmul(
                    out=h1_ps[:], lhsT=_mm(ones_t[:], use_f32r),
                    rhs=_mm(bc0_t[:], use_f32r), start=True, stop=False,
                )
                for k in range(HID // P):
                    tr_ps = h0psp.tile([P, P], F32, tag="trp")
                    nc.tensor.transpose(
                        out=tr_ps[:], in_=da_t[:, k * P:(k + 1) * P],
                        identity=ident_t[:],
                    )
                    daT_t = selp.tile([P, P], F32, tag="daT")
                    nc.vector.tensor_copy(out=daT_t[:], in_=tr_ps[:])
                    nc.tensor.matmul(
                        out=h1_ps[:], lhsT=_mm(daT_t[:], use_f32r),
                        rhs=_mm(wc0_t[k], use_f32r),
                        start=False, stop=(k == HID // P - 1),
                    )
                # hs1 = relu(h1) * out_norm  (rows = S1 nodes)
                nc.scalar.activation(
                    hs1_t[h], h1_ps[:], AF.Relu, scale=on1_t[:, h:h + 1]
                )

            # ---- stage B: layer 2 on the 8 agent rows ----
            AG = AGENTS_PER_CORE
            agg2_ps = mlppsp.tile([AG, HID], F32, tag="mlp")
            for h in range(NHALF):
                nc.tensor.matmul(
                    out=agg2_ps[:], lhsT=_mm(a2t_t[h], use_f32r),
                    rhs=_mm(hs1_t[h], use_f32r),
                    start=(h == 0), stop=(h == NHALF - 1),
                )
            da2_t = stp.tile([AG, HID], F32)
            nc.scalar.activation(
                da2_t[:], agg2_ps[:], AF.Copy, scale=in2_t[:, 0:1]
            )
            h2_ps = mlppsp.tile([AG, HID], F32, tag="mlp")
            nc.tensor.matmul(
                out=h2_ps[:], lhsT=_mm(ones_t[:1, :AG], use_f32r),
                rhs=_mm(bc1_t[:], use_f32r), start=True, stop=False,
            )
            for k in range(HID // P):
                tr_ps = h0psp.tile([P, AG], F32, tag="trp2")
                nc.tensor.transpose(
                    out=tr_ps[:, :AG], in_=da2_t[:, k * P:(k + 1) * P],
                    identity=ident_t[:AG, :AG],
                )
                da2T_t = selp.tile([P, AG], F32, tag="da2T")
                nc.vector.tensor_copy(out=da2T_t[:], in_=tr_ps[:])
                nc.tensor.matmul(
                    out=h2_ps[:], lhsT=_mm(da2T_t[:], use_f32r),
                    rhs=_mm(wc1_t[k], use_f32r),
                    start=False, stop=(k == HID // P - 1),
                )
            h2_t = stp.tile([AG, HID], F32)
            nc.scalar.activation(h2_t[:], h2_ps[:], AF.Relu)

            out_ps = mlppsp.tile([AG, EMB], F32, tag="mlp")
            nc.tensor.matmul(
                out=out_ps[:], lhsT=_mm(ones_t[:1, :AG], use_f32r),
                rhs=_mm(bemb_t[:], use_f32r), start=True, stop=False,
            )
            for k in range(HID // P):
                tr_ps = h0psp.tile([P, AG], F32, tag="trp2")
                nc.tensor.transpose(
                    out=tr_ps[:, :AG], in_=h2_t[:, k * P:(k + 1) * P],
                    identity=ident_t[:AG, :AG],
                )
                h2T_t = selp.tile([P, AG], F32, tag="da2T")
                nc.vector.tensor_copy(out=h2T_t[:], in_=tr_ps[:])
                nc.tensor.matmul(
                    out=out_ps[:], lhsT=_mm(h2T_t[:], use_f32r),
                    rhs=_mm(wemb_t[k], use_f32r),
                    start=False, stop=(k == HID // P - 1),
                )
            out_t = stp.tile([AG, EMB], F32)
            nc.scalar.activation(out_t[:], out_ps[:], AF.Copy)
            nc.sync.dma_start(out=out[:], in_=out_t[:])
    return nc


def prepare_inputs(x, src, dst):
    """Host-side integer index preprocessing + sharding. Returns
    (in_maps_percore_part, nch_per_half)."""
    deg_out = np.bincount(src, minlength=TOTAL).astype(np.float32)
    deg_in = np.bincount(dst, minlength=TOTAL).astype(np.float32)

    g = dst // NPG                     # graph id of each edge's dst
    is_agent = (dst % NPG) == 0

    cores = []
    nch_needed = 1
    for c in range(NCORES):
        # --- layer-2 edge bucket: dst is an agent owned by this core ---
        m2 = is_agent & (g >= c * AGENTS_PER_CORE) & (g < (c + 1) * AGENTS_PER_CORE)
        e2_src = src[m2]
        e2_ag = (g[m2] - c * AGENTS_PER_CORE).astype(np.int64)
        s1 = np.unique(e2_src)
        m1c = s1.size
        assert m1c <= NHALF * 127, f"S1 overflow: {m1c}"
        # slot: round-robin across halves, skipping slot 127 (pad/trash)
        order = np.arange(m1c)
        half = order % NHALF
        slot = half * P + order // NHALF
        # lookup: global node id -> slot
        loc = np.full(TOTAL, -1, dtype=np.int64)
        loc[s1] = slot
        a2t = np.zeros((M1, AGENTS_PER_CORE), dtype=np.float32)
        np.add.at(a2t, (loc[e2_src], e2_ag), 1.0)

        indeg1 = np.zeros(M1, np.float32)
        outdeg1 = np.zeros(M1, np.float32)
        indeg1[loc[s1]] = deg_in[s1]
        outdeg1[loc[s1]] = deg_out[s1]
        agents = (np.arange(AGENTS_PER_CORE) + c * AGENTS_PER_CORE) * NPG
        indeg2 = deg_in[agents].reshape(AGENTS_PER_CORE, 1)

        # --- layer-1 edge bucket: dst in S1 ---
        dl = loc[dst]
        sel = dl >= 0
        e1_src = src[sel]
        e1_slot = dl[sel]
        halves = []
        for h in range(NHALF):
            hm = (e1_slot // P) == h
            halves.append((e1_src[hm], e1_slot[hm] - h * P))
            nch_needed = max(nch_needed, -(-halves[h][0].size // P))
        cores.append(dict(a2t=a2t, indeg1=indeg1.reshape(NHALF, P).T,
                          outdeg1=outdeg1.reshape(NHALF, P).T,
                          indeg2=indeg2, halves=halves))
    return cores, deg_out, nch_needed


def pack_core(core, x, deg_out, nch_per_half):
    nchunk = NHALF * nch_per_half
    ne = nchunk * P
    xe = np.zeros((ne, IN_DIM), dtype=np.float32)
    odeg_e = np.zeros(ne, dtype=np.float32)
    dstl_e = np.full(ne, P - 1, dtype=np.float32)  # pads -> trash slot 127
    for h, (hsrc, hslot) in enumerate(core["halves"]):
        base = h * nch_per_half * P
        k = hsrc.size
        xe[base:base + k] = x[hsrc]
        odeg_e[base:base + k] = deg_out[hsrc]
        dstl_e[base:base + k] = hslot
    # [128, ...] layouts: edge e -> (e % 128, e // 128)
    return dict(
        xeT=np.ascontiguousarray(xe.T),
        odeg_e=np.ascontiguousarray(odeg_e.reshape(nchunk, P).T),
        dstl_e=np.ascontiguousarray(dstl_e.reshape(nchunk, P).T),
        indeg1=np.ascontiguousarray(core["indeg1"]),
        outdeg1=np.ascontiguousarray(core["outdeg1"]),
        indeg2=core["indeg2"],
        a2t=core["a2t"],
    )


def kernel(x, src, dst, num_nodes, nodes_per_graph,
           w_lin, b_lin, w_c0, b_c0, w_c1, b_c1, w_emb, b_emb,
           _debug=None) -> np.ndarray:
    from concourse.bass_utils import run_bass_kernel_spmd

    x = np.asarray(x, dtype=np.float32)
    src = np.asarray(src).astype(np.int64)
    dst = np.asarray(dst).astype(np.int64)
    assert int(num_nodes) == TOTAL and int(nodes_per_graph) == NPG
    assert x.shape == (TOTAL, IN_DIM)

    cores, deg_out, nch_per_half = prepare_inputs(x, src, dst)

    shared = dict(
        wlin=np.asarray(w_lin, np.float32),
        blin=np.asarray(b_lin, np.float32).reshape(1, HID),
        wc0=np.asarray(w_c0, np.float32),
        bc0=np.asarray(b_c0, np.float32).reshape(1, HID),
        wc1=np.asarray(w_c1, np.float32),
        bc1=np.asarray(b_c1, np.float32).reshape(1, HID),
        wemb=np.asarray(w_emb, np.float32),
        bemb=np.asarray(b_emb, np.float32).reshape(1, EMB),
        iota_in=np.broadcast_to(
            np.arange(P, dtype=np.float32), (P, P)).copy(),
    )
    in_maps = []
    for c in range(NCORES):
        m = pack_core(cores[c], x, deg_out, nch_per_half)
        m.update(shared)
        in_maps.append(m)

    nc = build_program(nch_per_half)
    core_ids = list(range(NCORES))
    if _debug is not None:
        _debug["nc"] = nc
        _debug["in_maps"] = in_maps
        _debug["nch_per_half"] = nch_per_half
    res = run_bass_kernel_spmd(nc, in_maps, core_ids)
    return np.concatenate([res.results[c]["out"] for c in range(NCORES)], axis=0)


# revision 12
# speedup vs baseline: 1.0008x; 1.0008x over previous
"""Trainium2 Bass kernel for nn_AggregateGCN (3-layer GCN, batched graph,
agent-node readout).

Math (reference): deg-normalized GraphConv x2 on top of a linear+relu input
projection, then a final projection of the 64 agent rows (nodes 0, N, 2N, ...).
Only the 64 agent rows of the last conv are read, so the exact computation
is the backward dependency cone:
  layer2 needs edges into the 64 agents (~2k edges -> ~2k distinct sources S1)
  layer1 needs edges into S1 (~64k edges), with per-edge h0 = relu(x@w_lin+b)
Degrees (in/out over ALL 4M edges) feed the symmetric normalization; the
host extracts integer degree counts + edge buckets (index-only preprocessing),
all float math runs on device.

Sharding: core c owns agents 8c..8c+7 and its full (replicated) cone ->
zero cross-device traffic; host concatenates the per-core [8, 64] outputs.

On device per core:
  - per-edge h0 rows via matmul chunks (lhsT = host-transposed x_e^T slice)
  - SpMM via selection-matrix matmuls accumulated in PSUM: S[e,d]=(dst_e==d)
  - degree norms folded in as per-partition ACT scales; biases as K=1
    rank-1 matmuls initializing the PSUM accumulation
"""
import sys

sys.path.insert(0, "/opt/trn_rl_repo")

import numpy as np
import concourse.bass as bass
import concourse.bacc as bacc
import concourse.mybir as mybir
import concourse.tile as tile
from concourse.masks import make_identity

F32 = mybir.dt.float32
F32R = mybir.dt.float32r
AF = mybir.ActivationFunctionType
ALU = mybir.AluOpType

# problem constants (fixed by the spec)
B = 64          # graphs
NPG = 2048      # nodes per graph
TOTAL = B * NPG
IN_DIM = 128
HID = 256
EMB = 64
NCORES = 8
AGENTS_PER_CORE = B // NCORES      # 8
M1 = 384                           # padded S1 slots per core (3 halves of 128)
NHALF = M1 // 128                  # 3
P = 128

USE_F32R = False  # f32r needs producer-side rounding; see notes


def _mk(use_f32r):
    def _mm(ap):
        return ap.bitcast(F32R) if use_f32r else ap
    return _mm


def build_program(nch_per_half: int, use_f32r: bool = USE_F32R,
                  repeat: int = 1) -> bass.Bass:
    """One SPMD program; per-core data differs via in_maps. repeat>1 re-runs
    the whole compute (including DMA loads) for slope-based HW timing."""
    nchunk = NHALF * nch_per_half
    ne = nchunk * P
    _mm = _mk(use_f32r)

    nc = bacc.Bacc(
        "TRN2", target_bir_lowering=False, debug=False, num_devices=NCORES
    )
    # per-core inputs
    xeT = nc.declare_dram_parameter("xeT", [P, ne], F32, isOutput=False)
    odeg_e = nc.declare_dram_parameter("odeg_e", [P, nchunk], F32, isOutput=False)
    dstl_e = nc.declare_dram_parameter("dstl_e", [P, nchunk], F32, isOutput=False)
    indeg1 = nc.declare_dram_parameter("indeg1", [P, NHALF], F32, isOutput=False)
    outdeg1 = nc.declare_dram_parameter("outdeg1", [P, NHALF], F32, isOutput=False)
    indeg2 = nc.declare_dram_parameter("indeg2", [AGENTS_PER_CORE, 1], F32, isOutput=False)
    a2t = nc.declare_dram_parameter("a2t", [M1, AGENTS_PER_CORE], F32, isOutput=False)
    # replicated weights
    wlin = nc.declare_dram_parameter("wlin", [IN_DIM, HID], F32, isOutput=False)
    blin = nc.declare_dram_parameter("blin", [P, HID], F32, isOutput=False)
    wc0 = nc.declare_dram_parameter("wc0", [HID, HID], F32, isOutput=False)
    bc0 = nc.declare_dram_parameter("bc0", [P, HID], F32, isOutput=False)
    wc1 = nc.declare_dram_parameter("wc1", [HID, HID], F32, isOutput=False)
    bc1 = nc.declare_dram_parameter("bc1", [P, HID], F32, isOutput=False)
    wemb = nc.declare_dram_parameter("wemb", [HID, EMB], F32, isOutput=False)
    bemb = nc.declare_dram_parameter("bemb", [P, EMB], F32, isOutput=False)
    iota_in = nc.declare_dram_parameter("iota_in", [P, P], F32, isOutput=False)
    out = nc.declare_dram_parameter("out", [AGENTS_PER_CORE, EMB], F32, isOutput=True)

    with tile.TileContext(nc) as tc:
        with (
            tc.tile_pool(name="const", bufs=1) as cp,
            tc.tile_pool(name="hs0p", bufs=4) as hs0p,
            tc.tile_pool(name="selp", bufs=4) as selp,
            tc.tile_pool(name="stage", bufs=1) as stp,
            tc.tile_pool(name="h0ps", bufs=2, space="PSUM") as h0psp,
            tc.tile_pool(name="aggps", bufs=2, space="PSUM") as aggpsp,
            tc.tile_pool(name="trps", bufs=2, space="PSUM") as trpsp,
            tc.tile_pool(name="mlpps", bufs=1, space="PSUM") as mlppsp,
            # PSUM budget: h0(2) + agg(2) + tr(2) + mlp(1) = 7 of 8 banks
        ):
            for _rep in range(repeat):
                emit_compute(
                    nc, cp, hs0p, selp, stp, h0psp, aggpsp, trpsp, mlppsp,
                    _mm, nch_per_half, nchunk, ne,
                    xeT, odeg_e, dstl_e, indeg1, outdeg1, indeg2, a2t,
                    wlin, blin, wc0, bc0, wc1, bc1, wemb, bemb, iota_in, out,
                )
    nc.compile()
    return nc


def emit_compute(nc, cp, hs0p, selp, stp, h0psp, aggpsp, trpsp, mlppsp,
                 _mm, nch_per_half, nchunk, ne,
                 xeT, odeg_e, dstl_e, indeg1, outdeg1, indeg2, a2t,
                 wlin, blin, wc0, bc0, wc1, bc1, wemb, bemb, iota_in, out):
    AG = AGENTS_PER_CORE
    # ---- constants / per-core metadata ----
    xeT_t = cp.tile([P, ne], F32, tag="xeT")
    nc.sync.dma_start(out=xeT_t[:], in_=xeT[:])
    wlin_t = cp.tile([IN_DIM, HID], F32, tag="wlin")
    nc.sync.dma_start(out=wlin_t[:], in_=wlin[:])
    wc0_t = cp.tile([P, HID // P, HID], F32, tag="wc0")
    nc.sync.dma_start(out=wc0_t[:], in_=wc0.rearrange("(c p) n -> p c n", p=P))
    wc1_t = cp.tile([P, HID // P, HID], F32, tag="wc1")
    nc.sync.dma_start(out=wc1_t[:], in_=wc1.rearrange("(c p) n -> p c n", p=P))
    wemb_t = cp.tile([P, HID // P, EMB], F32, tag="wemb")
    nc.sync.dma_start(out=wemb_t[:], in_=wemb.rearrange("(c p) n -> p c n", p=P))
    blin_t = cp.tile([P, HID], F32, tag="blin")
    nc.sync.dma_start(out=blin_t[:], in_=blin[:])
    bc0_t = cp.tile([P, HID], F32, tag="bc0")
    nc.sync.dma_start(out=bc0_t[:], in_=bc0[:])
    bc1_t = cp.tile([P, HID], F32, tag="bc1")
    nc.sync.dma_start(out=bc1_t[:], in_=bc1[:])
    bemb_t = cp.tile([P, EMB], F32, tag="bemb")
    nc.sync.dma_start(out=bemb_t[:], in_=bemb[:])
    iota_t = cp.tile([P, P], F32, tag="iota")
    nc.sync.dma_start(out=iota_t[:], in_=iota_in[:])
    dstl_t = cp.tile([P, nchunk], F32, tag="dstl")
    nc.sync.dma_start(out=dstl_t[:], in_=dstl_e[:])
    a2t_t = cp.tile([P, NHALF, AG], F32, tag="a2t")
    nc.sync.dma_start(out=a2t_t[:], in_=a2t.rearrange("(c p) n -> p c n", p=P))

    ident_t = cp.tile([P, P], F32, tag="ident")
    make_identity(nc, ident_t[:])

    # ---- degree -> norm scales (rsqrt(max(d,1))) ----
    def rsqrt_of(dram_ap, shape, tag):
        t = cp.tile(shape, F32, tag=tag)
        nc.sync.dma_start(out=t[:], in_=dram_ap)
        nc.vector.tensor_scalar(
            out=t[:], in0=t[:], scalar1=1.0, scalar2=None, op0=ALU.max
        )
        nc.vector.reciprocal(out=t[:], in_=t[:])
        nc.scalar.activation(t[:], t[:], AF.Sqrt)
        return t

    se_t = rsqrt_of(odeg_e[:], [P, nchunk], "se")          # per-edge out_norm
    in1_t = rsqrt_of(indeg1[:], [P, NHALF], "in1")         # per-S1-slot in_norm
    on1_t = rsqrt_of(outdeg1[:], [P, NHALF], "on1")        # per-S1-slot out_norm
    in2_t = rsqrt_of(indeg2[:], [AG, 1], "in2")            # per-agent in_norm

    # ---- stage A: per half, accumulate agg1 then h1 ----
    hs1_t = stp.tile([P, NHALF, HID], F32, tag="hs1")  # node-major, to stage B
    for h in range(NHALF):
        agg_ps = aggpsp.tile([P, HID], F32, tag="agg")
        for j in range(nch_per_half):
            c = h * nch_per_half + j
            h0_ps = h0psp.tile([P, HID], F32, tag="h0")
            nc.tensor.matmul(
                out=h0_ps[:],
                lhsT=_mm(xeT_t[:, c * P:(c + 1) * P]),
                rhs=_mm(wlin_t[:]),
                start=True, stop=True,
            )
            hb_t = hs0p.tile([P, HID], F32, tag="hb")
            nc.vector.tensor_add(out=hb_t[:], in0=h0_ps[:], in1=blin_t[:])
            hs0_t = hs0p.tile([P, HID], F32, tag="hs0")
            nc.scalar.activation(
                hs0_t[:], hb_t[:], AF.Relu, scale=se_t[:, c:c + 1]
            )
            s_t = selp.tile([P, P], F32, tag="sel")
            nc.vector.tensor_tensor(
                out=s_t[:],
                in0=dstl_t[:, c:c + 1].to_broadcast([P, P]),
                in1=iota_t[:],
                op=ALU.is_equal,
            )
            nc.tensor.matmul(
                out=agg_ps[:], lhsT=_mm(s_t[:]), rhs=_mm(hs0_t[:]),
                start=(j == 0), stop=(j == nch_per_half - 1),
            )
        # da = in_norm * agg  (row scale), then transpose for next matmul
        da_t = hs0p.tile([P, HID], F32, tag="da")
        nc.scalar.activation(
            da_t[:], agg_ps[:], AF.Copy, scale=in1_t[:, h:h + 1]
        )
        h1_ps = mlppsp.tile([P, HID], F32, tag="mlp")
        for k in range(HID // P):
            tr_ps = trpsp.tile([P, P], F32, tag="tr")
            nc.tensor.transpose(
                out=tr_ps[:], in_=da_t[:, k * P:(k + 1) * P],
                identity=ident_t[:],
            )
            daT_t = selp.tile([P, P], F32, tag="daT")
            nc.vector.tensor_copy(out=daT_t[:], in_=tr_ps[:])
            nc.tensor.matmul(
                out=h1_ps[:], lhsT=_mm(daT_t[:]), rhs=_mm(wc0_t[:, k, :]),
                start=(k == 0), stop=(k == HID // P - 1),
            )
        # hs1 = relu(h1 + bc0) * out_norm  (rows = S1 nodes)
        h1b_t = hs0p.tile([P, HID], F32, tag="hb")
        nc.vector.tensor_add(out=h1b_t[:], in0=h1_ps[:], in1=bc0_t[:])
        nc.scalar.activation(
            hs1_t[:, h, :], h1b_t[:], AF.Relu, scale=on1_t[:, h:h + 1]
        )

    # ---- stage B: layer 2 on the 8 agent rows ----
    agg2_ps = mlppsp.tile([AG, HID], F32, tag="mlp")
    for h in range(NHALF):
        nc.tensor.matmul(
            out=agg2_ps[:], lhsT=_mm(a2t_t[:, h, :]), rhs=_mm(hs1_t[:, h, :]),
            start=(h == 0), stop=(h == NHALF - 1),
        )
    da2_t = stp.tile([AG, HID], F32, tag="da2")
    nc.scalar.activation(
        da2_t[:], agg2_ps[:], AF.Copy, scale=in2_t[:, 0:1]
    )
    h2_ps = mlppsp.tile([AG, HID], F32, tag="mlp")
    for k in range(HID // P):
        tr_ps = trpsp.tile([P, AG], F32, tag="tr")
        nc.tensor.transpose(
            out=tr_ps[:, :AG], in_=da2_t[:, k * P:(k + 1) * P],
            identity=ident_t[:AG, :AG],
        )
        da2T_t = selp.tile([P, AG], F32, tag="da2T")
        nc.vector.tensor_copy(out=da2T_t[:], in_=tr_ps[:])
        nc.tensor.matmul(
            out=h2_ps[:], lhsT=_mm(da2T_t[:]), rhs=_mm(wc1_t[:, k, :]),
            start=(k == 0), stop=(k == HID // P - 1),
        )
    h2b_t = stp.tile([AG, HID], F32, tag="h2b")
    nc.vector.tensor_add(out=h2b_t[:], in0=h2_ps[:], in1=bc1_t[:AG, :])
    h2_t = stp.tile([AG, HID], F32, tag="h2")
    nc.scalar.activation(h2_t[:], h2b_t[:], AF.Relu)

    out_ps = mlppsp.tile([AG, EMB], F32, tag="mlp")
    for k in range(HID // P):
        tr_ps = trpsp.tile([P, AG], F32, tag="tr")
        nc.tensor.transpose(
            out=tr_ps[:, :AG], in_=h2_t[:, k * P:(k + 1) * P],
            identity=ident_t[:AG, :AG],
        )
        h2T_t = selp.tile([P, AG], F32, tag="da2T")
        nc.vector.tensor_copy(out=h2T_t[:], in_=tr_ps[:])
        nc.tensor.matmul(
            out=out_ps[:], lhsT=_mm(h2T_t[:]), rhs=_mm(wemb_t[:, k, :]),
            start=(k == 0), stop=(k == HID // P - 1),
        )
    out_t = stp.tile([AG, EMB], F32, tag="outt")
    nc.vector.tensor_add(out=out_t[:], in0=out_ps[:], in1=bemb_t[:AG, :])
    nc.sync.dma_start(out=out[:], in_=out_t[:])


def prepare_inputs(x, src, dst):
    """Host-side integer index preprocessing + sharding."""
    deg_out = np.bincount(src, minlength=TOTAL).astype(np.float32)
    deg_in = np.bincount(dst, minlength=TOTAL).astype(np.float32)

    g = dst // NPG                     # graph id of each edge's dst
    is_agent = (dst % NPG) == 0

    cores = []
    nch_needed = 1
    for c in range(NCORES):
        # --- layer-2 edge bucket: dst is an agent owned by this core ---
        m2 = is_agent & (g >= c * AGENTS_PER_CORE) & (g < (c + 1) * AGENTS_PER_CORE)
        e2_src = src[m2]
        e2_ag = (g[m2] - c * AGENTS_PER_CORE).astype(np.int64)
        s1 = np.unique(e2_src)
        m1c = s1.size
        assert m1c <= NHALF * 127, f"S1 overflow: {m1c}"
        # slot: round-robin across halves, skipping slot 127 (pad/trash)
        order = np.arange(m1c)
        half = order % NHALF
        slot = half * P + order // NHALF
        # lookup: global node id -> slot
        loc = np.full(TOTAL, -1, dtype=np.int64)
        loc[s1] = slot
        a2t = np.zeros((M1, AGENTS_PER_CORE), dtype=np.float32)
        np.add.at(a2t, (loc[e2_src], e2_ag), 1.0)

        indeg1 = np.zeros(M1, np.float32)
        outdeg1 = np.zeros(M1, np.float32)
        indeg1[loc[s1]] = deg_in[s1]
        outdeg1[loc[s1]] = deg_out[s1]
        agents = (np.arange(AGENTS_PER_CORE) + c * AGENTS_PER_CORE) * NPG
        indeg2 = deg_in[agents].reshape(AGENTS_PER_CORE, 1)

        # --- layer-1 edge bucket: dst in S1 ---
        dl = loc[dst]
        sel = dl >= 0
        e1_src = src[sel]
        e1_slot = dl[sel]
        halves = []
        for h in range(NHALF):
            hm = (e1_slot // P) == h
            halves.append((e1_src[hm], e1_slot[hm] - h * P))
            nch_needed = max(nch_needed, -(-halves[h][0].size // P))
        cores.append(dict(a2t=a2t, indeg1=indeg1.reshape(NHALF, P).T,
                          outdeg1=outdeg1.reshape(NHALF, P).T,
                          indeg2=indeg2, halves=halves))
    return cores, deg_out, nch_needed


def pack_core(core, x, deg_out, nch_per_half):
    nchunk = NHALF * nch_per_half
    ne = nchunk * P
    xe = np.zeros((ne, IN_DIM), dtype=np.float32)
    odeg_e = np.zeros(ne, dtype=np.float32)
    dstl_e = np.full(ne, P - 1, dtype=np.float32)  # pads -> trash slot 127
    for h, (hsrc, hslot) in enumerate(core["halves"]):
        base = h * nch_per_half * P
        k = hsrc.size
        xe[base:base + k] = x[hsrc]
        odeg_e[base:base + k] = deg_out[hsrc]
        dstl_e[base:base + k] = hslot
    # [128, ...] layouts: edge e -> (e % 128, e // 128)
    return dict(
        xeT=np.ascontiguousarray(xe.T),
        odeg_e=np.ascontiguousarray(odeg_e.reshape(nchunk, P).T),
        dstl_e=np.ascontiguousarray(dstl_e.reshape(nchunk, P).T),
        indeg1=np.ascontiguousarray(core["indeg1"]),
        outdeg1=np.ascontiguousarray(core["outdeg1"]),
        indeg2=core["indeg2"],
        a2t=core["a2t"],
    )


def shared_inputs(w_lin, b_lin, w_c0, b_c0, w_c1, b_c1, w_emb, b_emb):
    def bb(b, n):
        return np.ascontiguousarray(np.broadcast_to(
            np.asarray(b, np.float32).reshape(1, n), (P, n)))

    return dict(
        wlin=np.asarray(w_lin, np.float32),
        blin=bb(b_lin, HID),
        wc0=np.asarray(w_c0, np.float32),
        bc0=bb(b_c0, HID),
        wc1=np.asarray(w_c1, np.float32),
        bc1=bb(b_c1, HID),
        wemb=np.asarray(w_emb, np.float32),
        bemb=bb(b_emb, EMB),
        iota_in=np.broadcast_to(
            np.arange(P, dtype=np.float32), (P, P)).copy(),
    )


def kernel(x, src, dst, num_nodes, nodes_per_graph,
           w_lin, b_lin, w_c0, b_c0, w_c1, b_c1, w_emb, b_emb,
           _debug=None) -> np.ndarray:
    from concourse.bass_utils import run_bass_kernel_spmd

    x = np.asarray(x, dtype=np.float32)
    src = np.asarray(src).astype(np.int64)
    dst = np.asarray(dst).astype(np.int64)
    assert int(num_nodes) == TOTAL and int(nodes_per_graph) == NPG
    assert x.shape == (TOTAL, IN_DIM)

    cores, deg_out, nch_per_half = prepare_inputs(x, src, dst)

    shared = shared_inputs(w_lin, b_lin, w_c0, b_c0, w_c1, b_c1,
                           w_emb, b_emb)
    in_maps = []
    for c in range(NCORES):
        m = pack_core(cores[c], x, deg_out, nch_per_half)
        m.update(shared)
        in_maps.append(m)

    nc = build_program(nch_per_half)
    core_ids = list(range(NCORES))
    if _debug is not None:
        _debug["nc"] = nc
        _debug["in_maps"] = in_maps
        _debug["nch_per_half"] = nch_per_half
    res = run_bass_kernel_spmd(nc, in_maps, core_ids)
    return np.concatenate([res.results[c]["out"] for c in range(NCORES)], axis=0)
